# revision 1
# baseline (speedup 1.0000x reference)
"""LightGCN layer on 8 TRN2 NeuronCores.

out[r] = dis[r] * sum_{e: row_e = r} dis[col_e] * x[col_e]
where dis = masked rsqrt of destination-degree (deg = bincount(row)).

Strategy (edges sharded by destination chunk; xhat replicated):
- Host computes deg/dis from edge_index alone and ships xhat = dis*x as bf16
  rows padded to 256 bytes (dma_gather needs 256B-aligned elements).
- Edges are grouped by 128-row destination chunk. Chunks are dealt to the 8
  cores serpentine-by-count so the shared SPMD schedule is tight. Within each
  group of 8 chunks, edges form one run per bank (int16 gather indices
  address one of four 32768-row banks of xhat), chunks packed back-to-back
  with no per-chunk padding; only the run tail is padded to a 128-edge
  subtile.
- Device: per group, batched dma_gather instructions (<=1024 indices each,
  the SWDGE ring cap) pull xhat rows for whole runs. Subtiles may span chunk
  boundaries, so the static matmul plan emits one matmul per (subtile,
  candidate chunk) pair, where the candidate window is the union over cores
  of the chunks each core's edges occupy in that subtile. Each matmul's
  [128,128] one-hot comes from one DVE tensor_scalar is_equal against a
  host-packed per-plan-entry row-lo column (255 marks edges outside the
  entry's chunk, including pads). Chunk segment sums accumulate in PSUM; on
  completion the Activation engine scales by dis[row] into a group output
  buffer stored with one DMA (bf16, widened on host).
- Host scatters the 8 shards back through the chunk permutation and trims.
"""
import sys

try:
    import concourse  # noqa: F401  (provided by the booted axon site)
except ImportError:
    sys.path.insert(0, "/opt/trn_rl_repo")

import numpy as np
import ml_dtypes

N_NODES = 100000
N_EDGES = 1600000
D = 64
P = 128                       # edges per subtile (matmul contraction dim)
C = 128                       # destination rows per chunk (PSUM tile height)
XW = 128                      # padded xhat row width (bf16 -> 256B elements)
NCORE = 8
CPC = 98                      # chunks per core
NCHUNK = NCORE * CPC          # 784 chunks of 128 rows
NPAD = NCHUNK * C             # 100352 padded node count
BANKS = 4
BKSZ = 32768                  # int16-addressable rows per bank
G = 8                         # chunks per device group
NGRP = -(-CPC // G)           # 13 (last group has 2 chunks)
NI_SUB = 8                    # max subtiles per dma_gather (1024 idx ring cap)
PAD_RLO = 255.0               # one-hot-miss marker


def _groups():
    return [(g * G, min((g + 1) * G, CPC)) for g in range(NGRP)]


class _Plan:
    """Static shared schedule: runs, gather instructions, matmul plan."""

    def __init__(self, run_subs, run_start, nsub, w0, wn, mbase, n_mm,
                 entries):
        self.run_subs = run_subs      # [NGRP, BANKS] subtiles per run
        self.run_start = run_start    # [NGRP, BANKS] first subtile of run
        self.nsub = nsub
        self.w0 = w0                  # [nsub] first candidate chunk (loc j)
        self.wn = wn                  # [nsub] candidate count
        self.mbase = mbase            # [nsub] plan index of first candidate
        self.n_mm = n_mm
        self.entries = entries        # per chunk j: [(b, s, m), ...]

    def instructions(self):
        out = []
        for g in range(NGRP):
            for b in range(BANKS):
                o = int(self.run_start[g, b])
                e = o + int(self.run_subs[g, b])
                while o < e:
                    n = min(NI_SUB, e - o)
                    out.append((o, n))
                    o += n
        return out


def _make_plan(cnt_kjb):
    """Build the shared run/window/matmul plan from per-core per-chunk
    per-bank edge counts."""
    run_subs = np.zeros((NGRP, BANKS), np.int64)
    for g, (j0, j1) in enumerate(_groups()):
        run_edges = cnt_kjb[:, j0:j1, :].sum(axis=1)         # [NCORE, BANKS]
        run_subs[g] = np.maximum(1, -(-run_edges // P)).max(axis=0)
    flat = np.zeros(NGRP * BANKS + 1, np.int64)
    np.cumsum(run_subs.reshape(-1), out=flat[1:])
    run_start = flat[:-1].reshape(NGRP, BANKS)
    nsub = int(flat[-1])

    w0 = np.zeros(nsub, np.int64)
    wn = np.zeros(nsub, np.int64)
    for g, (j0, j1) in enumerate(_groups()):
        for b in range(BANKS):
            rs, ns = int(run_start[g, b]), int(run_subs[g, b])
            cs = np.zeros((NCORE, j1 - j0 + 1), np.int64)
            np.cumsum(cnt_kjb[:, j0:j1, b], axis=1, out=cs[:, 1:])
            lo = np.full(ns, j1 - j0, np.int64)
            hi = np.full(ns, -1, np.int64)
            for k in range(NCORE):
                tot = int(cs[k, -1])
                if tot == 0:
                    continue
                nsk = -(-tot // P)
                s = np.arange(nsk)
                lo_k = np.searchsorted(cs[k, 1:], s * P, side="right")
                last = np.minimum((s + 1) * P, tot) - 1
                hi_k = np.searchsorted(cs[k, 1:], last, side="right")
                lo[:nsk] = np.minimum(lo[:nsk], lo_k)
                hi[:nsk] = np.maximum(hi[:nsk], hi_k)
            empty = hi < lo
            lo[empty], hi[empty] = 0, 0
            w0[rs:rs + ns] = j0 + lo
            wn[rs:rs + ns] = hi - lo + 1

    mb = np.zeros(nsub + 1, np.int64)
    np.cumsum(wn, out=mb[1:])
    n_mm = int(mb[-1])

    entries = [[] for _ in range(CPC)]
    for g, (j0, j1) in enumerate(_groups()):
        for b in range(BANKS):
            rs, ns = int(run_start[g, b]), int(run_subs[g, b])
            for s in range(rs, rs + ns):
                for w in range(int(wn[s])):
                    j = int(w0[s]) + w
                    entries[j].append((b, s, int(mb[s]) + w))
    for j in range(CPC):
        entries[j].sort()
        assert entries[j], f"chunk {j} has no plan entries"
    return _Plan(run_subs, run_start, nsub, w0, wn, mb[:-1], n_mm, entries)


def _pack_edges(row, col):
    chunk = (row >> 7).astype(np.int64)
    bank = (col >> 15).astype(np.int64)
    cnt4 = np.bincount(chunk * BANKS + bank,
                       minlength=NCHUNK * BANKS).reshape(NCHUNK, BANKS)

    # serpentine-deal chunks to cores by total count (aligns run lengths)
    order = np.argsort(-cnt4.sum(1), kind="stable")
    i = np.arange(NCHUNK)
    rnd, pos = i // NCORE, i % NCORE
    corei = np.where(rnd % 2 == 0, pos, NCORE - 1 - pos)
    core_of = np.empty(NCHUNK, np.int64)
    loc_of = np.empty(NCHUNK, np.int64)
    core_of[order] = corei
    loc_of[order] = rnd

    cnt_kjb = np.zeros((NCORE, CPC, BANKS), np.int64)
    cnt_kjb[core_of, loc_of] = cnt4

    # greedy refinement: swapping the chunks of two cores at the same slot
    # only changes that group's runs — accept swaps that reduce the group's
    # shared subtile count (sum over banks of ceil(max-over-cores run / P))
    gidx = [np.arange(j0, j1) for (j0, j1) in _groups()]
    chunk_at = np.zeros((NCORE, CPC), np.int64)
    chunk_at[core_of, loc_of] = np.arange(NCHUNK)

    def gscore(g):
        run = cnt_kjb[:, gidx[g], :].sum(axis=1)             # [NCORE, BANKS]
        return int((-(-run.max(axis=0) // P)).sum())

    for _ in range(3):
        improved = False
        for j in range(CPC):
            g = j // G
            base = gscore(g)
            for k1 in range(NCORE):
                for k2 in range(k1 + 1, NCORE):
                    r1 = cnt_kjb[k1, j].copy()
                    r2 = cnt_kjb[k2, j].copy()
                    cnt_kjb[k1, j], cnt_kjb[k2, j] = r2, r1
                    new = gscore(g)
                    if new < base:
                        base = new
                        improved = True
                        c1, c2 = chunk_at[k1, j], chunk_at[k2, j]
                        chunk_at[k1, j], chunk_at[k2, j] = c2, c1
                        core_of[c1], core_of[c2] = k2, k1
                    else:
                        cnt_kjb[k1, j], cnt_kjb[k2, j] = r1, r2
        if not improved:
            break

    plan = _make_plan(cnt_kjb)
    nsub, n_mm = plan.nsub, plan.n_mm

    # per-edge placement: rank within (core, run), runs in (group, bank)
    # order, chunks in slot order inside each run
    k_e = core_of[chunk]
    j_e = loc_of[chunk]
    g_e = j_e // G
    run_e = g_e * BANKS + bank                               # [E]
    nrun = NGRP * BANKS
    key = (k_e * nrun + run_e) * G + (j_e - g_e * G)
    eorder = np.argsort(key, kind="stable")
    ks = key[eorder]
    kr = ks // G                                             # (core, run)
    starts = np.searchsorted(kr, np.arange(NCORE * nrun), side="left")
    r = np.arange(len(ks)) - starts[kr]
    run_start_flat = plan.run_start.reshape(-1)
    s_e = run_start_flat[kr % nrun] + r // P
    p_e = r % P
    ke = kr // nrun

    j_loc = j_e[eorder]
    m_e = plan.mbase[s_e] + (j_loc - plan.w0[s_e])
    assert (j_loc >= plan.w0[s_e]).all()
    assert (j_loc < plan.w0[s_e] + plan.wn[s_e]).all()

    idxs = np.zeros((NCORE, nsub, P), np.int16)
    rlom = np.full((NCORE, P, n_mm), PAD_RLO, np.float32)
    idxs[ke, s_e, p_e] = (col[eorder] & (BKSZ - 1)).astype(np.int16)
    rlom[ke, p_e, m_e] = (row[eorder] & (C - 1)).astype(np.float32)

    # int16 wrap per gather instruction: idx j at [j % 16, j // 16]
    idxw16 = np.empty((NCORE, 16, nsub * P // 16), np.int16)
    for (o, n) in plan.instructions():
        blk = idxs[:, o:o + n, :].reshape(NCORE, n * P // 16, 16)
        idxw16[:, :, o * 8:(o + n) * 8] = np.swapaxes(blk, 1, 2)
    idxw = np.broadcast_to(idxw16[:, None, :, :],
                           (NCORE, 8, 16, nsub * P // 16))
    idxw = np.ascontiguousarray(idxw).reshape(NCORE, 128, nsub * P // 16)

    return plan, idxw, rlom, core_of, loc_of


def _dma_gather_raw(gp, mybir, out_ap, in_ap, idxs_ap, num_idxs, elem_size,
                    elem_step):
    """dma_gather with elem_size below the 256B transpose-mode minimum.
    The non-transpose ucode accepts any elem_size as long as the row stride
    (elem_step) is a multiple of 256B; verified on hardware."""
    stride_bytes = elem_step * mybir.dt.size(in_ap.dtype)
    assert stride_bytes % 256 == 0
    _in_ap = gp.lower_ap_dma(in_ap, for_custom_bir_dma=True)
    _idxs_ap = gp.lower_ap(idxs_ap)
    _out_ap = gp.lower_ap(out_ap)
    return gp.add_instruction(
        mybir.InstDMAGatherAnt(
            name=gp.bass.get_next_instruction_name(),
            ins=[*_in_ap, _idxs_ap,
                 gp.lower_val_access(gp.to_reg(num_idxs))],
            outs=[_out_ap],
            transpose=False,
            num_idxs=num_idxs,
            elem_size=elem_size,
            stride_bytes_256=stride_bytes // 256,
            gen_mode=0,
            single_packet=True,
            queue_num=0,
            sbuf_tokens_per_rank=0,
            sbuf_free_dim_per_rank=0,
            sbuf_free_dim_pad_per_rank=0,
            sbuf_byte_offset=0,
        ))


def _build_program(plan, sim=False):
    import concourse.bacc as bacc
    import concourse.mybir as mybir
    import concourse.tile as tile
    from concourse.library_config import mlp

    f32, bf16 = mybir.dt.float32, mybir.dt.bfloat16
    i16 = mybir.dt.int16
    Alu = mybir.AluOpType

    nsub, n_mm = plan.nsub, plan.n_mm
    nc = bacc.Bacc("TRN2", target_bir_lowering=False, debug=False,
                   enable_asserts=False, num_devices=1 if sim else NCORE)

    t_x = nc.dram_tensor("xpad", [N_NODES, XW], bf16, kind="ExternalInput")
    t_idxw = nc.dram_tensor("idxw", [128, nsub * P // 16], i16,
                            kind="ExternalInput")
    t_rlom = nc.dram_tensor("rlom", [P, n_mm], mybir.dt.uint8,
                            kind="ExternalInput")
    t_iota = nc.dram_tensor("iota", [P, C], bf16, kind="ExternalInput")
    t_disl = nc.dram_tensor("disl", [C, CPC], f32, kind="ExternalInput")
    t_out = nc.dram_tensor("out", [C, CPC, D], bf16, kind="ExternalOutput")

    bank_hi = [min((b + 1) * BKSZ, N_NODES) for b in range(BANKS)]

    with tile.TileContext(nc) as tc:
        with (
            tc.tile_pool(name="const", bufs=1) as cpool,
            tc.tile_pool(name="iw", bufs=3) as ipool,
            tc.tile_pool(name="gt", bufs=24) as gpool,
            tc.tile_pool(name="oh", bufs=224) as ohpool,
            tc.tile_pool(name="ob", bufs=2) as opool,
            tc.tile_pool(name="psum", bufs=8, space="PSUM") as ppool,
        ):
            nc.gpsimd.load_library(mlp)
            iota = cpool.tile([P, C], bf16, tag="iota")
            disl = cpool.tile([C, CPC], f32, tag="disl")
            rlom8 = cpool.tile([P, n_mm], mybir.dt.uint8, tag="rlom8")
            rlom = cpool.tile([P, n_mm], f32, tag="rlom")

            for g, (j0, j1) in enumerate(_groups()):
                s0 = int(plan.run_start[g, 0])
                s1 = int(plan.run_start[g, BANKS - 1]
                         + plan.run_subs[g, BANKS - 1])
                iw = ipool.tile([128, (s1 - s0) * 8], i16, tag="iw")
                nc.scalar.dma_start(out=iw[:],
                                    in_=t_idxw.ap()[:, s0 * 8:s1 * 8])

                slot_map = {}
                for b in range(BANKS):
                    rs = int(plan.run_start[g, b])
                    re = rs + int(plan.run_subs[g, b])
                    o = rs
                    while o < re:
                        n = min(NI_SUB, re - o)
                        gt = gpool.tile([P, NI_SUB, D], bf16, tag="gt")
                        _dma_gather_raw(
                            nc.gpsimd, mybir, gt[:, 0:n, :],
                            t_x.ap()[b * BKSZ:bank_hi[b]],
                            iw[:, (o - s0) * 8:(o - s0 + n) * 8],
                            num_idxs=n * P, elem_size=D, elem_step=XW)
                        for t in range(n):
                            slot_map[o + t] = (gt, t)
                        o += n

                if g == 0:
                    # constants load behind the first gathers so the DMA
                    # engines start on the critical gather stream immediately
                    nc.sync.dma_start(out=iota[:], in_=t_iota.ap())
                    nc.scalar.dma_start(out=disl[:], in_=t_disl.ap())
                    nc.sync.dma_start(out=rlom8[:], in_=t_rlom.ap())
                    nc.vector.tensor_copy(out=rlom[:], in_=rlom8[:])

                ob = opool.tile([C, G, D], bf16, tag="ob")
                for j in range(j0, j1):
                    pm = ppool.tile([C, D], f32, tag="pm", space="PSUM")
                    tl = plan.entries[j]
                    for ti, (b, s, m) in enumerate(tl):
                        gt, sl = slot_map[s]
                        oh = ohpool.tile([P, C], bf16, tag="oh")
                        nc.vector.tensor_scalar(oh[:], iota[:],
                                                rlom[:, m:m + 1], None,
                                                Alu.is_equal)
                        nc.tensor.matmul(pm[:], lhsT=oh[:],
                                         rhs=gt[:, sl, :],
                                         start=(ti == 0),
                                         stop=(ti == len(tl) - 1))
                    nc.scalar.activation(ob[:, j - j0, :], pm[:],
                                         mybir.ActivationFunctionType.Copy,
                                         scale=disl[:, j:j + 1])
                nw = j1 - j0
                nc.sync.dma_start(out=t_out.ap()[:, j0:j0 + nw, :],
                                  in_=ob[:, :nw, :])

    nc.compile()
    return nc


def _prepare(row, col):
    plan, idxw, rlom, core_of, loc_of = _pack_edges(row, col)
    deg = np.bincount(row, minlength=N_NODES).astype(np.float32)
    dis = np.where(deg > 0, 1.0 / np.sqrt(np.maximum(deg, 1.0)), 0.0)
    return {"build_args": (plan,), "plan": plan, "idxw": idxw, "rlom": rlom,
            "core_of": core_of, "loc_of": loc_of,
            "dis": dis.astype(np.float32)}


def _in_maps(x, prep):
    dis = prep["dis"]
    xpad = np.zeros((N_NODES, XW), ml_dtypes.bfloat16)
    xpad[:, :D] = (x * dis[:, None]).astype(ml_dtypes.bfloat16)
    iota = np.tile(np.arange(C, dtype=ml_dtypes.bfloat16), (P, 1))
    dis_pad = np.zeros(NPAD, np.float32)
    dis_pad[:N_NODES] = dis
    gmap = np.zeros((NCORE, CPC), np.int64)
    gmap[prep["core_of"], prep["loc_of"]] = np.arange(NCHUNK)
    disl = dis_pad.reshape(NCHUNK, C)[gmap]                  # [NCORE, CPC, C]
    disl = np.ascontiguousarray(np.transpose(disl, (0, 2, 1)))
    return [
        {"xpad": xpad, "idxw": prep["idxw"][k],
         "rlom": prep["rlom"][k].astype(np.uint8),
         "iota": iota, "disl": disl[k]}
        for k in range(NCORE)
    ]


def kernel(x, edge_index):
    from concourse import bass_utils

    x = np.asarray(x, dtype=np.float32)
    ei = np.asarray(edge_index)
    row = ei[0].astype(np.int64)
    col = ei[1].astype(np.int64)

    prep = _prepare(row, col)
    nc = _build_program(*prep["build_args"])
    in_maps = _in_maps(x, prep)
    res = bass_utils.run_bass_kernel_spmd(nc, in_maps, core_ids=list(range(NCORE)))

    gmap = np.zeros((NCORE, CPC), np.int64)
    gmap[prep["core_of"], prep["loc_of"]] = np.arange(NCHUNK)
    out_pad = np.empty((NCHUNK, C, D), np.float32)
    for k in range(NCORE):
        out_pad[gmap[k]] = np.transpose(
            res.results[k]["out"].astype(np.float32), (1, 0, 2))
    return out_pad.reshape(NPAD, D)[:N_NODES].astype(np.float32)

